# revision 12
# baseline (speedup 1.0000x reference)
"""Multi-head attention Trainium2 kernel (Bass/Tile), SPMD over 8 cores.

Sharding: core c handles batch b = c//4 and heads h0 = (c%4)*4 .. h0+4.
Each core runs the SAME program; per-core inputs are pre-sliced on host.

Math per core (Lq = Lk = 2048, dk = dv = 64, 4 heads):
  qhT[h] = (Wq_h^T @ q^T)          [64, 2048]   (q transposed on-chip via PE)
  khT[h] = (Wk_h^T @ k^T)          [64, 2048]
  vh[h]  = v @ Wv_h + bv           [2048, 64]
  natural side (attn output):
    S = qhT^T @ khT (+ -1e8*mask, accumulated in PSUM via (-1e8*I) @ mask_f32)
    E = exp(S * srow)   srow = (1-pad)/8 per row  -> padded rows give E = 1
    Z = row sums (ACT accum_out);  attn = E * (1/Z)
  transposed side (for attn@V, no big transposes needed):
    St = khT^T @ qhT (+ mask^T term);  Et = exp(St/8)
    outT[h] = sum_kk vh^T-block @ Et    [64, 2048]  (unnormalized)
  output projection:
    out = sum_h diag(w_h) (outT[h]^T @ W0_h) + pad/2048 * C,  w_h = (1-pad)/Z_h
    C = sum_h colsum(vh_h)^T @ W0_h     (uniform-attention row for padded rows)
Host adds b0 and sums the per-core partial outputs per batch.
"""

import numpy as np
from contextlib import ExitStack

import concourse.bass as bass
import concourse.bacc as bacc
import concourse.mybir as mybir
import concourse.tile as tile

B, L, D, H, DK, DV = 2, 2048, 1024, 16, 64, 64
HPC = 4          # heads per core
NCORES = 8
NEG = -1.0e8
NI = L // 128    # 16 row blocks
NJ = L // 512    # 4 col tiles
VALID, MIXED, MASKED = 0, 1, 2

f32 = mybir.dt.float32
u8 = mybir.dt.uint8
Alu = mybir.AluOpType
Act = mybir.ActivationFunctionType


def _classify(m):
    """m: bool [B, L, L] -> per-tile class grid [NI][NJ] merged over batch."""
    g = np.zeros((NI, NJ), np.int32)
    for i in range(NI):
        for j in range(NJ):
            sub = m[:, i * 128:(i + 1) * 128, j * 512:(j + 1) * 512]
            if sub.all():
                g[i, j] = MASKED
            elif not sub.any():
                g[i, j] = VALID
            else:
                g[i, j] = MIXED
    return tuple(tuple(int(x) for x in row) for row in g)


_PROG_CACHE = {}
_LAST_RES = None
_RUN_KWARGS = {}


def build_program(natcls, tcls, skip_masked_writes=False):
    key = (natcls, tcls, skip_masked_writes)
    if key in _PROG_CACHE:
        return _PROG_CACHE[key]

    nc = bacc.Bacc("TRN2", target_bir_lowering=False, debug=False,
                   enable_asserts=False, num_devices=NCORES)

    ap = {}
    for name, shape, dt, kind in [
        ("q", [L, D], f32, "ExternalInput"),
        ("k", [L, D], f32, "ExternalInput"),
        ("v", [L, D], f32, "ExternalInput"),
        ("wq", [D, HPC * DK], f32, "ExternalInput"),
        ("wk", [D, HPC * DK], f32, "ExternalInput"),
        ("wv", [D, HPC * DV], f32, "ExternalInput"),
        ("w0", [HPC * DV, D], f32, "ExternalInput"),
        ("bq2", [128, 2], f32, "ExternalInput"),
        ("bk2", [128, 2], f32, "ExternalInput"),
        ("bvrow", [1, HPC * DV], f32, "ExternalInput"),
        ("srow", [128, NI], f32, "ExternalInput"),
        ("padf", [128, NI], f32, "ExternalInput"),
        ("pkeep", [128, NI], f32, "ExternalInput"),
        ("ident", [128, 128], f32, "ExternalInput"),
        ("negid", [128, 128], f32, "ExternalInput"),
        ("onesrow", [1, 128], f32, "ExternalInput"),
        ("onescol", [128, 1], f32, "ExternalInput"),
        ("mask", [L, L], u8, "ExternalInput"),
        ("maskt", [L, L], u8, "ExternalInput"),
        ("attn_part", [HPC, L, L], f32, "ExternalOutput"),
        ("out_part", [L, D], f32, "ExternalOutput"),
    ]:
        ap[name] = nc.dram_tensor(name, shape, dt, kind=kind).ap()

    # transposed-side bookkeeping from tcls
    t_mixed = [(i2, j2) for i2 in range(NI) for j2 in range(NJ)
               if tcls[i2][j2] == MIXED]
    preload_tmask = len(t_mixed) <= 24
    lasts = []
    for j2 in range(NJ):
        contrib = [i2 for i2 in range(NI) if tcls[i2][j2] != MASKED]
        lasts.append(contrib[-1] if contrib else None)

    with tile.TileContext(nc) as tc:
        with ExitStack() as root:
            res = root.enter_context(tc.tile_pool(name="res", bufs=1))

            # ---- resident constants / weights ----
            wq_sb = res.tile([128, 8, HPC * DK], f32)
            nc.sync.dma_start(out=wq_sb, in_=ap["wq"].rearrange("(c p) n -> p c n", p=128))
            wk_sb = res.tile([128, 8, HPC * DK], f32)
            nc.sync.dma_start(out=wk_sb, in_=ap["wk"].rearrange("(c p) n -> p c n", p=128))
            wv_sb = res.tile([128, 8, HPC * DV], f32)
            nc.sync.dma_start(out=wv_sb, in_=ap["wv"].rearrange("(c p) n -> p c n", p=128))
            w0_sb = res.tile([64, HPC, D], f32)
            nc.sync.dma_start(out=w0_sb, in_=ap["w0"].rearrange("(hh p) n -> p hh n", p=64))
            bq2_sb = res.tile([128, 2], f32)
            nc.sync.dma_start(out=bq2_sb, in_=ap["bq2"])
            bk2_sb = res.tile([128, 2], f32)
            nc.sync.dma_start(out=bk2_sb, in_=ap["bk2"])
            bvrow_sb = res.tile([1, HPC * DV], f32)
            nc.sync.dma_start(out=bvrow_sb, in_=ap["bvrow"])
            srow_sb = res.tile([128, NI], f32)
            nc.sync.dma_start(out=srow_sb, in_=ap["srow"])
            padf_sb = res.tile([128, NI], f32)
            nc.sync.dma_start(out=padf_sb, in_=ap["padf"])
            pkeep_sb = res.tile([128, NI], f32)
            nc.sync.dma_start(out=pkeep_sb, in_=ap["pkeep"])
            ident_sb = res.tile([128, 128], f32)
            nc.sync.dma_start(out=ident_sb, in_=ap["ident"])
            negid_sb = res.tile([128, 128], f32)
            nc.sync.dma_start(out=negid_sb, in_=ap["negid"])
            onesrow_sb = res.tile([1, 128], f32)
            nc.sync.dma_start(out=onesrow_sb, in_=ap["onesrow"])
            onescol_sb = res.tile([128, 1], f32)
            nc.sync.dma_start(out=onescol_sb, in_=ap["onescol"])
            ones512_sb = res.tile([128, 512], f32)
            nc.vector.memset(ones512_sb, 1.0)

            # ---- resident intermediates ----
            # qhT/khT: partition = (pair-local head)*64+dk, free = (hp, L)
            qhT_sb = res.tile([128, 2, L], f32)
            khT_sb = res.tile([128, 2, L], f32)
            vh_sb = res.tile([128, NI, HPC * DV], f32)   # [Lk-block, l, (h,dv)]
            oT_sb = res.tile([64, HPC, L], f32)          # raw out^T per head
            w_sb = res.tile([128, HPC * NI], f32)        # (1-pad)/Z per (h,i)
            csum_sb = res.tile([64, HPC], f32)
            C_sb = res.tile([1, D], f32)
            padsc_sb = res.tile([128, NI], f32)
            nc.vector.tensor_scalar_mul(padsc_sb, padf_sb, 1.0 / float(L))

            def qhT_lhsT(h, i):  # [64, 128]
                return qhT_sb[(h % 2) * 64:(h % 2 + 1) * 64, h // 2,
                              i * 128:(i + 1) * 128]

            def qhT_rhs(h, j):  # [64, 512]
                return qhT_sb[(h % 2) * 64:(h % 2 + 1) * 64, h // 2,
                              j * 512:(j + 1) * 512]

            def khT_lhsT(h, i):
                return khT_sb[(h % 2) * 64:(h % 2 + 1) * 64, h // 2,
                              i * 128:(i + 1) * 128]

            def khT_rhs(h, j):
                return khT_sb[(h % 2) * 64:(h % 2 + 1) * 64, h // 2,
                              j * 512:(j + 1) * 512]

            # ================= phase 1: transposes + projections =================
            with ExitStack() as p1:
                nat = p1.enter_context(tc.tile_pool(name="nat", bufs=2))
                tbl = p1.enter_context(tc.tile_pool(name="tbl", bufs=2))
                tps = p1.enter_context(tc.tile_pool(name="tps", bufs=2, space="PSUM"))
                pqk = p1.enter_context(tc.tile_pool(name="pqk", bufs=2, space="PSUM"))
                pvv = p1.enter_context(tc.tile_pool(name="pvv", bufs=2, space="PSUM"))

                for l in range(NI):
                    rows = slice(l * 128, (l + 1) * 128)
                    blocks = {}
                    for name in ("q", "k", "v"):
                        bn = nat.tile([128, D], f32, tag=f"n{name}")
                        nc.sync.dma_start(out=bn, in_=ap[name][rows, :])
                        bt = tbl.tile([128, 8, 128], f32, tag=f"t{name}")
                        for d in range(8):
                            tp = tps.tile([128, 128], f32)
                            nc.tensor.transpose(tp, bn[:, d * 128:(d + 1) * 128], ident_sb)
                            nc.vector.tensor_copy(bt[:, d, :], tp)
                        blocks[name] = bt
                    # q/k projections, two heads at a time
                    for wsb, bsb, dst, src in ((wq_sb, bq2_sb, qhT_sb, blocks["q"]),
                                               (wk_sb, bk2_sb, khT_sb, blocks["k"])):
                        for hp in range(2):
                            pq = pqk.tile([128, 128], f32)
                            for d in range(8):
                                nc.tensor.matmul(
                                    pq, wsb[:, d, hp * 128:(hp + 1) * 128],
                                    src[:, d, :], start=(d == 0), stop=(d == 7))
                            nc.scalar.add(dst[:, hp, rows], pq, add=bsb[:, hp:hp + 1])
                    # v projection, all 4 heads at once
                    pv = pvv.tile([128, HPC * DV], f32)
                    for d in range(8):
                        nc.tensor.matmul(pv, blocks["v"][:, d, :], wv_sb[:, d, :],
                                         start=(d == 0), stop=False)
                    nc.tensor.matmul(pv, onesrow_sb, bvrow_sb, start=False, stop=True)
                    nc.vector.tensor_copy(vh_sb[:, l, :], pv)

            # ================= phase 2: natural scores/softmax -> attn ===========
            with ExitStack() as p2:
                sps = p2.enter_context(tc.tile_pool(name="sps", bufs=2, space="PSUM"))
                epool = p2.enter_context(tc.tile_pool(name="epool", bufs=2))
                mpool = p2.enter_context(tc.tile_pool(name="mpool", bufs=4))
                zpool = p2.enter_context(tc.tile_pool(name="zpool", bufs=4))

                for i in range(NI):
                    rows = slice(i * 128, (i + 1) * 128)
                    mf = {}
                    for j in range(NJ):
                        if natcls[i][j] == MIXED:
                            t = mpool.tile([128, 512], f32)
                            nc.gpsimd.dma_start(
                                out=t, in_=ap["mask"][rows, j * 512:(j + 1) * 512])
                            mf[j] = t
                    # runs of consecutive computed tiles
                    runs = []
                    j = 0
                    while j < NJ:
                        if natcls[i][j] == MASKED:
                            j += 1
                            continue
                        j0 = j
                        while j < NJ and natcls[i][j] != MASKED:
                            j += 1
                        runs.append((j0, j))
                    for h in range(HPC):
                        ps = sps.tile([128, L], f32)
                        E = epool.tile([128, L], f32)
                        Zp = zpool.tile([128, NJ], f32, tag="zp")
                        nc.vector.memset(Zp, 0.0)
                        for j in range(NJ):
                            c = natcls[i][j]
                            cols = slice(j * 512, (j + 1) * 512)
                            if c == MASKED:
                                if not skip_masked_writes:
                                    nc.vector.tensor_scalar_mul(
                                        E[:, cols], ones512_sb, padf_sb[:, i:i + 1])
                                nc.vector.tensor_scalar_mul(
                                    Zp[:, j:j + 1], padf_sb[:, i:i + 1], 512.0)
                                continue
                            nc.tensor.matmul(ps[:, cols], qhT_lhsT(h, i), khT_rhs(h, j),
                                             start=True, stop=(c == VALID))
                            if c == MIXED:
                                nc.tensor.matmul(ps[:, cols], negid_sb, mf[j],
                                                 start=False, stop=True)
                        for (j0, j1) in runs:
                            span = slice(j0 * 512, j1 * 512)
                            nc.scalar.activation(E[:, span], ps[:, span], Act.Exp,
                                                 bias=0.0, scale=srow_sb[:, i:i + 1],
                                                 accum_out=Zp[:, j0:j0 + 1])
                        Zs = zpool.tile([128, 1], f32, tag="zs")
                        nc.vector.tensor_reduce(Zs, Zp, axis=mybir.AxisListType.X,
                                                op=Alu.add)
                        Zi = zpool.tile([128, 1], f32, tag="zi")
                        nc.vector.reciprocal(Zi, Zs)
                        nc.vector.tensor_mul(w_sb[:, h * NI + i:h * NI + i + 1],
                                             Zi, pkeep_sb[:, i:i + 1])
                        if skip_masked_writes:
                            for (j0, j1) in runs:
                                span = slice(j0 * 512, j1 * 512)
                                nc.gpsimd.tensor_scalar_mul(E[:, span], E[:, span], Zi)
                                nc.sync.dma_start(out=ap["attn_part"][h, rows, span],
                                                  in_=E[:, span])
                        else:
                            nc.gpsimd.tensor_scalar_mul(E, E, Zi)
                            nc.sync.dma_start(out=ap["attn_part"][h, rows, :], in_=E)

            # ================= phase 3: transposed side -> raw out^T =============
            with ExitStack() as p3:
                stp = p3.enter_context(tc.tile_pool(name="stp", bufs=2, space="PSUM"))
                otp = p3.enter_context(tc.tile_pool(name="otp", bufs=1, space="PSUM"))
                etp = p3.enter_context(tc.tile_pool(name="etp", bufs=3))
                mtp = p3.enter_context(
                    tc.tile_pool(name="mtp", bufs=max(1, len(t_mixed) if preload_tmask else 4)))

                mtf = {}
                if preload_tmask:
                    for (i2, j2) in t_mixed:
                        t = mtp.tile([128, 512], f32, tag="mt")
                        nc.gpsimd.dma_start(
                            out=t, in_=ap["maskt"][i2 * 128:(i2 + 1) * 128,
                                                   j2 * 512:(j2 + 1) * 512])
                        mtf[(i2, j2)] = t

                for h in range(HPC):
                    oT = otp.tile([64, L], f32)
                    started = [False] * NJ
                    for j2 in range(NJ):
                        if lasts[j2] is None:
                            nc.vector.memset(oT[:, j2 * 512:(j2 + 1) * 512], 0.0)
                    for i2 in range(NI):
                        # pairs of j2 -> spans of 1024 for fewer ACT ops
                        for jp in range(NJ // 2):
                            js = [2 * jp, 2 * jp + 1]
                            comp = [j2 for j2 in js if tcls[i2][j2] != MASKED]
                            if not comp:
                                continue
                            st = stp.tile([128, 1024], f32)
                            for j2 in comp:
                                off = (j2 - js[0]) * 512
                                cols = slice(off, off + 512)
                                c = tcls[i2][j2]
                                nc.tensor.matmul(st[:, cols], khT_lhsT(h, i2),
                                                 qhT_rhs(h, j2),
                                                 start=True, stop=(c == VALID))
                                if c == MIXED:
                                    if (i2, j2) in mtf:
                                        mt = mtf[(i2, j2)]
                                    else:
                                        mt = mtp.tile([128, 512], f32, tag="mt")
                                        nc.gpsimd.dma_start(
                                            out=mt,
                                            in_=ap["maskt"][i2 * 128:(i2 + 1) * 128,
                                                            j2 * 512:(j2 + 1) * 512])
                                    nc.tensor.matmul(st[:, cols], negid_sb, mt,
                                                     start=False, stop=True)
                            Et = etp.tile([128, 1024], f32)
                            if comp == js:  # contiguous pair -> one ACT
                                nc.scalar.activation(Et, st, Act.Exp,
                                                     bias=0.0, scale=0.125)
                            else:
                                for j2 in comp:
                                    off = (j2 - js[0]) * 512
                                    cols = slice(off, off + 512)
                                    nc.scalar.activation(Et[:, cols], st[:, cols],
                                                         Act.Exp, bias=0.0, scale=0.125)
                            for j2 in comp:
                                off = (j2 - js[0]) * 512
                                nc.tensor.matmul(
                                    oT[:, j2 * 512:(j2 + 1) * 512],
                                    vh_sb[:, i2, h * DV:(h + 1) * DV],
                                    Et[:, off:off + 512],
                                    start=(not started[j2]),
                                    stop=(i2 == lasts[j2]))
                                started[j2] = True
                    nc.vector.tensor_copy(oT_sb[:, h, :], oT)

            # ================= phase 4: output projection ========================
            with ExitStack() as p4:
                cbp = p4.enter_context(tc.tile_pool(name="cbp", bufs=1, space="PSUM"))
                spool = p4.enter_context(tc.tile_pool(name="spool", bufs=3))

                with ExitStack() as p4a:
                    cps = p4a.enter_context(
                        tc.tile_pool(name="cps", bufs=1, space="PSUM"))
                    # column sums of vh (for padded-row uniform output)
                    for h in range(HPC):
                        cp = cps.tile([64, 1], f32, tag="c")
                        for i2 in range(NI):
                            nc.tensor.matmul(cp, vh_sb[:, i2, h * DV:(h + 1) * DV],
                                             onescol_sb, start=(i2 == 0),
                                             stop=(i2 == NI - 1))
                        nc.vector.tensor_copy(csum_sb[:, h:h + 1], cp)
                    Cp = cps.tile([1, D], f32, tag="C")
                    for h in range(HPC):
                        for dj in range(2):
                            nc.tensor.matmul(Cp[:, dj * 512:(dj + 1) * 512],
                                             csum_sb[:, h:h + 1],
                                             w0_sb[:, h, dj * 512:(dj + 1) * 512],
                                             start=(h == 0), stop=(h == HPC - 1))
                    nc.vector.tensor_copy(C_sb, Cp)
                    Cb = cbp.tile([128, D], f32)
                    for dj in range(2):
                        nc.tensor.matmul(Cb[:, dj * 512:(dj + 1) * 512], onesrow_sb,
                                         C_sb[:, dj * 512:(dj + 1) * 512],
                                         start=True, stop=True)

                pop = p4.enter_context(tc.tile_pool(name="pop", bufs=4, space="PSUM"))

                for i in range(NI):
                    rows = slice(i * 128, (i + 1) * 128)
                    for dj in range(2):
                        cols = slice(dj * 512, (dj + 1) * 512)
                        po = [pop.tile([128, 512], f32, tag="po", name=f"po{hh}")
                              for hh in range(HPC)]
                        for h in range(HPC):
                            nc.tensor.matmul(po[h], oT_sb[:, h, rows],
                                             w0_sb[:, h, cols],
                                             start=True, stop=True)
                        t0 = spool.tile([128, 512], f32, tag="t0")
                        nc.vector.tensor_scalar_mul(t0, po[0],
                                                    w_sb[:, 0 * NI + i:0 * NI + i + 1])
                        t1 = spool.tile([128, 512], f32, tag="t1")
                        nc.vector.scalar_tensor_tensor(
                            t1, po[1], w_sb[:, 1 * NI + i:1 * NI + i + 1], t0,
                            op0=Alu.mult, op1=Alu.add)
                        t2 = spool.tile([128, 512], f32, tag="t0")
                        nc.vector.scalar_tensor_tensor(
                            t2, po[2], w_sb[:, 2 * NI + i:2 * NI + i + 1], t1,
                            op0=Alu.mult, op1=Alu.add)
                        t3 = spool.tile([128, 512], f32, tag="t1")
                        nc.vector.scalar_tensor_tensor(
                            t3, po[3], w_sb[:, 3 * NI + i:3 * NI + i + 1], t2,
                            op0=Alu.mult, op1=Alu.add)
                        t4 = spool.tile([128, 512], f32, tag="t0")
                        nc.vector.scalar_tensor_tensor(
                            t4, Cb[:, cols], padsc_sb[:, i:i + 1], t3,
                            op0=Alu.mult, op1=Alu.add)
                        nc.sync.dma_start(out=ap["out_part"][rows, cols], in_=t4)

    nc.compile()
    _PROG_CACHE[key] = nc
    return nc


def make_core_inputs(core, q, k, v, Wq, bq, Wk, bk, Wv, bv, W0,
                     key_padding_mask, attn_mask):
    b = core // (NCORES // B)
    h0 = (core % (NCORES // B)) * HPC
    cs = slice(h0 * DK, (h0 + HPC) * DK)
    pad = key_padding_mask[b].astype(np.float32)          # [L]
    srow = np.where(pad > 0, 0.0, 1.0 / np.sqrt(DK)).astype(np.float32)
    bq_s = bq[cs].astype(np.float32)
    bk_s = bk[cs].astype(np.float32)
    return {
        "q": np.ascontiguousarray(q[b], np.float32),
        "k": np.ascontiguousarray(k[b], np.float32),
        "v": np.ascontiguousarray(v[b], np.float32),
        "wq": np.ascontiguousarray(Wq[:, cs], np.float32),
        "wk": np.ascontiguousarray(Wk[:, cs], np.float32),
        "wv": np.ascontiguousarray(Wv[:, cs], np.float32),
        "w0": np.ascontiguousarray(W0[cs, :], np.float32),
        "bq2": np.ascontiguousarray(bq_s.reshape(2, 128).T),
        "bk2": np.ascontiguousarray(bk_s.reshape(2, 128).T),
        "bvrow": np.ascontiguousarray(bv[cs].astype(np.float32).reshape(1, -1)),
        "srow": np.ascontiguousarray(srow.reshape(NI, 128).T),
        "padf": np.ascontiguousarray(pad.reshape(NI, 128).T),
        "pkeep": np.ascontiguousarray((1.0 - pad).reshape(NI, 128).T),
        "ident": np.eye(128, dtype=np.float32),
        "negid": (NEG * np.eye(128)).astype(np.float32),
        "onesrow": np.ones((1, 128), np.float32),
        "onescol": np.ones((128, 1), np.float32),
        "mask": np.ascontiguousarray(attn_mask[b].astype(np.uint8)),
        "maskt": np.ascontiguousarray(attn_mask[b].T.astype(np.uint8)),
    }


def kernel(q, k, v, Wq, bq, Wk, bk, Wv, bv, W0, b0,
           key_padding_mask, attn_mask):
    q = np.asarray(q); k = np.asarray(k); v = np.asarray(v)
    Wq = np.asarray(Wq); Wk = np.asarray(Wk); Wv = np.asarray(Wv)
    bq = np.asarray(bq); bk = np.asarray(bk); bv = np.asarray(bv)
    W0 = np.asarray(W0); b0 = np.asarray(b0)
    key_padding_mask = np.asarray(key_padding_mask)
    attn_mask = np.asarray(attn_mask)

    natcls = _classify(attn_mask)
    tcls = _classify(np.swapaxes(attn_mask, 1, 2))
    nc = build_program(natcls, tcls, skip_masked_writes=False)

    in_maps = [make_core_inputs(c, q, k, v, Wq, bq, Wk, bk, Wv, bv, W0,
                                key_padding_mask, attn_mask)
               for c in range(NCORES)]

    from concourse.bass_utils import run_bass_kernel_spmd
    res = run_bass_kernel_spmd(nc, in_maps, list(range(NCORES)), **_RUN_KWARGS)
    global _LAST_RES
    _LAST_RES = res

    attn = np.empty((B, H, L, L), np.float32)
    out = np.zeros((B, L, D), np.float32)
    for c in range(NCORES):
        b = c // (NCORES // B)
        h0 = (c % (NCORES // B)) * HPC
        r = res.results[c]
        attn[b, h0:h0 + HPC] = r["attn_part"]
        out[b] += r["out_part"]
    out += b0.astype(np.float32)
    return out, attn
